# revision 5
# baseline (speedup 1.0000x reference)
"""Causal self-attention (B=4, T=1024, D=2048, H=16) on 8 trn2 NeuronCores.

Sharding: data-parallel over batch (4) x tensor-parallel over heads (2).
Core c handles batch b = c//2, head-half hh = c%2 (heads hh*8 .. hh*8+8).

Per-core plan (all matmuls float32r, fp32 PSUM accumulation):
  qT/kT  [d, t]  : lhsT = w_{q,k} tile [k,c], rhs = xT [k,t]
  v      [t, c]  : lhsT = xT tile [k,t], rhs = wv [k,c]
  sT     [tk,tq] : lhsT = kT block, rhs = qT slice (causal: tq >= 128*j only)
  pT     = exp(scale * sT) via ACT (no max-subtraction; |scaled scores| ~ 6)
  diag blocks masked multiplicatively with an upper-triangular 0/1 mask
  yT     [d, tq] += v_j-gemm: lhsT = v block, rhs = pT block (PSUM accum)
  r      [1, tq] += ones^T @ pT (softmax row sums, same rhs stream)
  yT_norm = yT * broadcast(1/r)  (PE broadcast of recip + DVE multiply)
  pairwise AllGather of yT across the 2 head-half cores
  out    [t, c_half] = yT_full-gemm against this half's w_proj columns
Host side: slice/transpose inputs per core, concat outputs (pure gather).
"""

import numpy as np

import concourse.bass as bass
import concourse.mybir as mybir
import concourse.tile as tile
from concourse import bacc
from concourse.bass_utils import run_bass_kernel_spmd

B, T, D = 4, 1024, 2048
H, DH = 16, 128
N_CORES = 8
TP = 2                      # head-halves per batch
HPC = H // TP               # heads per core = 8
CPC = HPC * DH              # channels per core = 1024
KC = D // 128               # contraction chunks = 16
SCALE = 1.0 / float(np.sqrt(DH))

F32 = mybir.dt.float32
F32R = mybir.dt.float32r

PAIRS = [[2 * i, 2 * i + 1] for i in range(B)]


def build_kernel():
    nc = bacc.Bacc("TRN2", target_bir_lowering=False, debug=False,
                   num_devices=N_CORES)

    xT_ap = nc.dram_tensor("xT", [D, T], F32R, kind="ExternalInput").ap()
    wq_ap = nc.dram_tensor("wq", [D, CPC], F32R, kind="ExternalInput").ap()
    wk_ap = nc.dram_tensor("wk", [D, CPC], F32R, kind="ExternalInput").ap()
    wv_ap = nc.dram_tensor("wv", [D, CPC], F32R, kind="ExternalInput").ap()
    wp_ap = nc.dram_tensor("wp", [D, CPC], F32R, kind="ExternalInput").ap()
    maskT_ap = nc.dram_tensor("maskT", [128, 128], F32R, kind="ExternalInput").ap()
    out_ap = nc.dram_tensor("out", [T, CPC], F32, kind="ExternalOutput").ap()

    with tile.TileContext(nc) as tc:
        _body(nc, tc, xT_ap, wq_ap, wk_ap, wv_ap, wp_ap, maskT_ap, out_ap)
    nc.compile()
    return nc


def _body(nc, tc, xT_ap, wq_ap, wk_ap, wv_ap, wp_ap, maskT_ap, out_ap):
    Exp = mybir.ActivationFunctionType.Exp
    mult = mybir.AluOpType.mult

    with tc.tile_pool(name="const", bufs=1) as const:
        maskT = const.tile([128, 128], F32R, tag="maskT")
        nc.sync.dma_start(out=maskT, in_=maskT_ap)
        ones_f32 = const.tile([128, 1], F32, tag="ones_f32")
        nc.vector.memset(ones_f32, 1.0)
        ones_col = const.tile([128, 1], F32R, tag="ones_col")
        nc.scalar.copy(out=ones_col, in_=ones_f32)
        ones_row_f32 = const.tile([1, 128], F32, tag="ones_row_f32")
        nc.vector.memset(ones_row_f32, 1.0)
        ones_row = const.tile([1, 128], F32R, tag="ones_row")
        nc.scalar.copy(out=ones_row, in_=ones_row_f32)

        with tc.tile_pool(name="dram", bufs=1, space="DRAM") as dram:
            yt_loc = dram.tile([CPC, T], F32R)
            yt_all = dram.tile([TP, CPC, T], F32R)

            with tc.tile_pool(name="yt", bufs=HPC) as yt_pool:
                yT = [yt_pool.tile([128, T], F32R, tag="yT",
                                   name=f"yT{h}") for h in range(HPC)]

                with tc.tile_pool(name="qkv", bufs=3 * HPC) as qkv_pool:
                    qT = [qkv_pool.tile([128, T], F32R, tag="qkv",
                                        name=f"qT{h}") for h in range(HPC)]
                    kT = [qkv_pool.tile([128, T], F32R, tag="qkv",
                                        name=f"kT{h}") for h in range(HPC)]
                    vv = [qkv_pool.tile([128, CPC], F32R, tag="qkv",
                                        name=f"vv{j}") for j in range(8)]

                    _phase_a(nc, tc, xT_ap, wq_ap, wk_ap, wv_ap, qT, kT, vv)
                    _phase_b(nc, tc, qT, kT, vv, yT, maskT, ones_col,
                             ones_row, Exp, mult)

                # yT -> DRAM, pairwise AllGather
                for h in range(HPC):
                    nc.sync.dma_start(out=yt_loc[128 * h:128 * (h + 1), :],
                                      in_=yT[h])
            nc.gpsimd.collective_compute(
                "AllGather", mybir.AluOpType.bypass, replica_groups=PAIRS,
                ins=[yt_loc.opt()], outs=[yt_all.opt()])

            _phase_c(nc, tc, yt_all, wp_ap, out_ap)


def _phase_a(nc, tc, xT_ap, wq_ap, wk_ap, wv_ap, qT, kT, vv):
    """qkv projections.  xT stays SBUF-resident; wq/wk/wv are read once."""
    with tc.tile_pool(name="xa", bufs=KC) as xa:
        xts = []
        for k in range(KC):
            xt = xa.tile([128, T], F32R, tag="xT", name=f"xt{k}")
            nc.sync.dma_start(out=xt, in_=xT_ap[128 * k:128 * (k + 1), :])
            xts.append(xt)

        # qT / kT: per head-chunk c the 16 w tiles stay resident so both
        # t-halves reuse them (w read exactly once from HBM).
        with tc.tile_pool(name="wqk", bufs=20) as wqk, \
             tc.tile_pool(name="pa", bufs=4, space="PSUM") as pa:
            for c in range(HPC):
                for (w_ap, outT) in ((wq_ap, qT[c]), (wk_ap, kT[c])):
                    wts = []
                    for k in range(KC):
                        wt = wqk.tile([128, 128], F32R, tag="wqk",
                                      name=f"wqk{c}_{k}")
                        nc.sync.dma_start(
                            out=wt,
                            in_=w_ap[128 * k:128 * (k + 1),
                                     128 * c:128 * (c + 1)])
                        wts.append(wt)
                    for th in range(2):
                        ps = pa.tile([128, 512], F32, tag="pqk")
                        for k in range(KC):
                            nc.tensor.matmul(
                                ps, wts[k], xts[k][:, 512 * th:512 * (th + 1)],
                                start=(k == 0), stop=(k == KC - 1))
                        nc.scalar.copy(out=outT[:, 512 * th:512 * (th + 1)],
                                       in_=ps)

        # v natural [t, c]: stationary xT slices, moving wv (read once).
        with tc.tile_pool(name="wv", bufs=3) as wvp, \
             tc.tile_pool(name="pv", bufs=8, space="PSUM") as pv:
            for ch in range(2):
                ps = [pv.tile([128, 512], F32, tag="pv", name=f"pv{ch}_{i}")
                      for i in range(8)]
                for k in range(KC):
                    wt = wvp.tile([128, 512], F32R, tag="wv",
                                  name=f"wv{ch}_{k}")
                    nc.sync.dma_start(
                        out=wt,
                        in_=wv_ap[128 * k:128 * (k + 1),
                                  512 * ch:512 * (ch + 1)])
                    for tch in range(8):
                        nc.tensor.matmul(
                            ps[tch], xts[k][:, 128 * tch:128 * (tch + 1)], wt,
                            start=(k == 0), stop=(k == KC - 1))
                for tch in range(8):
                    nc.scalar.copy(out=vv[tch][:, 512 * ch:512 * (ch + 1)],
                                   in_=ps[tch])


def _phase_b(nc, tc, qT, kT, vv, yT, maskT, ones_col, ones_row, Exp, mult):
    """Attention per head: scores^T -> exp -> AV + row sums -> normalize."""
    with tc.tile_pool(name="pt", bufs=10) as ptp, \
         tc.tile_pool(name="att_sm", bufs=2) as asm, \
         tc.tile_pool(name="att_bc", bufs=3) as abc, \
         tc.tile_pool(name="ps_s", bufs=2, space="PSUM") as pss, \
         tc.tile_pool(name="ps_y", bufs=2, space="PSUM") as psy, \
         tc.tile_pool(name="ps_r", bufs=2, space="PSUM") as psr, \
         tc.tile_pool(name="ps_b", bufs=1, space="PSUM") as psb:
        for h in range(HPC):
            pts = []
            for j in range(8):
                pt = ptp.tile([128, T], F32R, tag="pT", name=f"pT{h}_{j}")
                pts.append(pt)
                off = 128 * j
                while off < T:
                    cw = min(512, T - off)
                    sp = pss.tile([128, 512], F32, tag="sT")
                    nc.tensor.matmul(
                        sp[:, :cw], kT[h][:, 128 * j:128 * (j + 1)],
                        qT[h][:, off:off + cw], start=True, stop=True)
                    nc.scalar.activation(
                        out=pt[:, off - 128 * j:off - 128 * j + cw],
                        in_=sp[:, :cw], func=Exp, scale=SCALE)
                    off += cw
                # causal mask on the diagonal block (tile-local cols 0:128)
                nc.vector.tensor_tensor(out=pt[:, 0:128], in0=pt[:, 0:128],
                                        in1=maskT, op=mult)

            for g in range(2):
                tq0 = 512 * g
                jmax = 4 * (g + 1)
                yp = psy.tile([128, 512], F32, tag="yp")
                rp = psr.tile([1, 512], F32, tag="rp")
                for j in range(jmax):
                    lo = max(tq0, 128 * j)          # first valid tq
                    w = tq0 + 512 - lo
                    rhs = pts[j][:, lo - 128 * j:lo - 128 * j + w]
                    vblk = vv[j][:, 128 * h:128 * (h + 1)]
                    nc.tensor.matmul(yp[:, lo - tq0:lo - tq0 + w], vblk, rhs,
                                     start=(j == 0), stop=(j == jmax - 1))
                    nc.tensor.matmul(rp[:, lo - tq0:lo - tq0 + w], ones_col,
                                     rhs, start=(j == 0), stop=(j == jmax - 1))
                # softmax denominator -> reciprocal -> PE broadcast -> mult
                r_sb = asm.tile([1, 512], F32, tag="r_sb")
                nc.scalar.copy(out=r_sb, in_=rp)
                rec = asm.tile([1, 512], F32, tag="rec")
                nc.vector.reciprocal(out=rec, in_=r_sb)
                rec_r = asm.tile([1, 512], F32R, tag="rec_r")
                nc.scalar.copy(out=rec_r, in_=rec)
                bp = psb.tile([128, 512], F32, tag="bp")
                nc.tensor.matmul(bp, ones_row, rec_r, start=True, stop=True)
                bc = abc.tile([128, 512], F32, tag="bc")
                nc.scalar.copy(out=bc, in_=bp)
                nc.vector.tensor_tensor(out=yT[h][:, tq0:tq0 + 512],
                                        in0=yp, in1=bc, op=mult)


def _phase_c(nc, tc, yt_all, wp_ap, out_ap):
    """Output projection out[t, c_half] = yT_full.T-gemm @ wp columns."""
    with tc.tile_pool(name="peer", bufs=2 * HPC) as peer_pool, \
         tc.tile_pool(name="wp", bufs=4) as wpp, \
         tc.tile_pool(name="out_sb", bufs=4) as osb, \
         tc.tile_pool(name="ps_o", bufs=8, space="PSUM") as pso:
        # Both gathered halves are loaded so the program is core-independent
        # (which half is "mine" differs per core; SPMD must not branch).
        yfull = []
        for r in range(TP):
            for h2 in range(HPC):
                t2 = peer_pool.tile([128, T], F32R, tag="yfull",
                                    name=f"yfull{r}_{h2}")
                nc.sync.dma_start(
                    out=t2, in_=yt_all[r, 128 * h2:128 * (h2 + 1), :])
                yfull.append(t2)

        for cc in range(2):          # 512-wide halves of my CPC out cols
            ps = [pso.tile([128, 512], F32, tag="po", name=f"po{cc}_{m}")
                  for m in range(8)]
            for kk in range(KC):
                wt = wpp.tile([128, 512], F32R, tag="wp",
                              name=f"wp{cc}_{kk}")
                nc.sync.dma_start(
                    out=wt, in_=wp_ap[128 * kk:128 * (kk + 1),
                                      512 * cc:512 * (cc + 1)])
                for m in range(8):
                    nc.tensor.matmul(
                        ps[m], yfull[kk][:, 128 * m:128 * (m + 1)], wt,
                        start=(kk == 0), stop=(kk == KC - 1))
            for m in range(8):
                ot = osb.tile([128, 512], F32, tag="ot")
                nc.scalar.copy(out=ot, in_=ps[m])
                nc.sync.dma_start(
                    out=out_ap[128 * m:128 * (m + 1),
                               512 * cc:512 * (cc + 1)],
                    in_=ot)


_NC_CACHE = None


def _get_nc():
    global _NC_CACHE
    if _NC_CACHE is None:
        _NC_CACHE = build_kernel()
    return _NC_CACHE


def kernel(x, w_qkv, w_proj, _trace=False, _trace_kwargs=None):
    x = np.asarray(x, dtype=np.float32)
    w_qkv = np.asarray(w_qkv, dtype=np.float32)
    w_proj = np.asarray(w_proj, dtype=np.float32)

    maskT = np.triu(np.ones((128, 128), dtype=np.float32))

    in_maps = []
    for c in range(N_CORES):
        b, hh = c // TP, c % TP
        cols = slice(hh * CPC, (hh + 1) * CPC)
        in_maps.append({
            "xT": np.ascontiguousarray(x[b].T),
            "wq": np.ascontiguousarray(w_qkv[:, :D][:, cols]),
            "wk": np.ascontiguousarray(w_qkv[:, D:2 * D][:, cols]),
            "wv": np.ascontiguousarray(w_qkv[:, 2 * D:][:, cols]),
            "wp": np.ascontiguousarray(w_proj[:, cols]),
            "maskT": maskT,
        })

    nc = _get_nc()
    res = run_bass_kernel_spmd(nc, in_maps, list(range(N_CORES)),
                               trace=_trace, **(_trace_kwargs or {}))

    out = np.empty((B, T, D), dtype=np.float32)
    for c in range(N_CORES):
        b, hh = c // TP, c % TP
        out[b, :, hh * CPC:(hh + 1) * CPC] = res.results[c]["out"]
    if _trace:
        return out, res
    return out


# revision 7
# speedup vs baseline: 1.5871x; 1.5871x over previous
"""Causal self-attention (B=4, T=1024, D=2048, H=16) on 8 trn2 NeuronCores.

Sharding: data-parallel over batch (4) x tensor-parallel over heads (2).
Core c handles batch b = c//2, head-half hh = c%2 (heads hh*8 .. hh*8+8).

Per-core plan (all matmuls float32r, fp32 PSUM accumulation):
  qT/kT  [d, t]  : lhsT = w_{q,k} tile [k,c], rhs = xT [k,t]
  v      [t, c]  : lhsT = xT tile [k,t], rhs = wv [k,c]
  sT     [tk,tq] : lhsT = kT block, rhs = qT slice (causal: tq >= 128*j only)
  pT     = exp(scale * sT) via ACT (no max-subtraction; |scaled scores| ~ 6)
  diag blocks masked multiplicatively with an upper-triangular 0/1 mask
  yT     [d, tq] += v_j-gemm: lhsT = v block, rhs = pT block (PSUM accum)
  r      [1, tq] += ones^T @ pT (softmax row sums, same rhs stream)
  yT_norm = yT * bcast(1/r) (DVE copy -> GpSimd partition bcast -> DVE)
  per-head pairwise AllGather of yT (overlaps later heads' compute)
  out    [t, c_half] = yT_full-gemm against this half's w_proj columns
Host side: slice/transpose inputs per core, concat outputs (pure gather).
"""

import numpy as np

import concourse.bass as bass
import concourse.mybir as mybir
import concourse.tile as tile
from concourse import bacc
from concourse.bass_utils import run_bass_kernel_spmd

B, T, D = 4, 1024, 2048
H, DH = 16, 128
N_CORES = 8
TP = 2                      # head-halves per batch
HPC = H // TP               # heads per core = 8
CPC = HPC * DH              # channels per core = 1024
KC = D // 128               # contraction chunks = 16
SCALE = 1.0 / float(np.sqrt(DH))

F32 = mybir.dt.float32
F32R = mybir.dt.float32r

PAIRS = [[2 * i, 2 * i + 1] for i in range(B)]


def build_kernel():
    nc = bacc.Bacc("TRN2", target_bir_lowering=False, debug=False,
                   num_devices=N_CORES)

    xT_ap = nc.dram_tensor("xT", [D, T], F32R, kind="ExternalInput").ap()
    wq_ap = nc.dram_tensor("wq", [D, CPC], F32R, kind="ExternalInput").ap()
    wk_ap = nc.dram_tensor("wk", [D, CPC], F32R, kind="ExternalInput").ap()
    wv_ap = nc.dram_tensor("wv", [D, CPC], F32R, kind="ExternalInput").ap()
    wp_ap = nc.dram_tensor("wp", [D, CPC], F32R, kind="ExternalInput").ap()
    maskT_ap = nc.dram_tensor("maskT", [128, 128], F32R, kind="ExternalInput").ap()
    out_ap = nc.dram_tensor("out", [T, CPC], F32, kind="ExternalOutput").ap()

    with tile.TileContext(nc) as tc:
        _body(nc, tc, xT_ap, wq_ap, wk_ap, wv_ap, wp_ap, maskT_ap, out_ap)
    nc.compile()
    return nc


def _body(nc, tc, xT_ap, wq_ap, wk_ap, wv_ap, wp_ap, maskT_ap, out_ap):
    with tc.tile_pool(name="const", bufs=1) as const:
        maskT = const.tile([128, 128], F32R, tag="maskT")
        nc.sync.dma_start(out=maskT, in_=maskT_ap)
        ones_f32 = const.tile([128, 1], F32, tag="ones_f32")
        nc.vector.memset(ones_f32, 1.0)
        ones_col = const.tile([128, 1], F32R, tag="ones_col")
        nc.scalar.copy(out=ones_col, in_=ones_f32)

        with tc.tile_pool(name="dram", bufs=HPC, space="DRAM") as dram:
            yt_loc = [dram.tile([128, T], F32R, tag="ytl", name=f"ytl{h}")
                      for h in range(HPC)]
            yt_all = [dram.tile([TP, 128, T], F32R, tag="yta", name=f"yta{h}")
                      for h in range(HPC)]

            with tc.tile_pool(name="qkv", bufs=3 * HPC) as qkv_pool:
                qT = [qkv_pool.tile([128, T], F32R, tag="qkv",
                                    name=f"qT{h}") for h in range(HPC)]
                kT = [qkv_pool.tile([128, T], F32R, tag="qkv",
                                    name=f"kT{h}") for h in range(HPC)]
                vv = [qkv_pool.tile([128, CPC], F32R, tag="qkv",
                                    name=f"vv{j}") for j in range(8)]

                _phase_a(nc, tc, xT_ap, wq_ap, wk_ap, wv_ap, qT, kT, vv)
                _phase_b(nc, tc, qT, kT, vv, maskT, ones_col, yt_loc, yt_all)

            _phase_c(nc, tc, yt_all, wp_ap, out_ap)


def _phase_a(nc, tc, xT_ap, wq_ap, wk_ap, wv_ap, qT, kT, vv):
    """qkv projections.  xT stays SBUF-resident; wq/wk/wv are read once."""
    with tc.tile_pool(name="xa", bufs=KC) as xa:
        xts = []
        for k in range(KC):
            xt = xa.tile([128, T], F32R, tag="xT", name=f"xt{k}")
            nc.sync.dma_start(out=xt, in_=xT_ap[128 * k:128 * (k + 1), :])
            xts.append(xt)

        # qT / kT: one DMA per (c, w) loads all 16 stationary k-tiles; both
        # t-halves reuse them (w read exactly once from HBM).
        with tc.tile_pool(name="wqk", bufs=3) as wqk, \
             tc.tile_pool(name="pa", bufs=4, space="PSUM") as pa:
            for c in range(HPC):
                for (w_ap, outT) in ((wq_ap, qT[c]), (wk_ap, kT[c])):
                    wt = wqk.tile([128, KC, 128], F32R, tag="wqk")
                    nc.scalar.dma_start(
                        out=wt,
                        in_=w_ap[:, 128 * c:128 * (c + 1)].rearrange(
                            "(k p) j -> p k j", p=128))
                    for th in range(2):
                        ps = pa.tile([128, 512], F32, tag="pqk")
                        for k in range(KC):
                            nc.tensor.matmul(
                                ps, wt[:, k, :],
                                xts[k][:, 512 * th:512 * (th + 1)],
                                start=(k == 0), stop=(k == KC - 1))
                        nc.scalar.copy(out=outT[:, 512 * th:512 * (th + 1)],
                                       in_=ps)

        # v natural [t, c]: stationary xT slices, moving wv (read once).
        with tc.tile_pool(name="wv", bufs=3) as wvp, \
             tc.tile_pool(name="pv", bufs=8, space="PSUM") as pv:
            for ch in range(2):
                ps = [pv.tile([128, 512], F32, tag="pv", name=f"pv{ch}_{i}")
                      for i in range(8)]
                for k in range(KC):
                    wt = wvp.tile([128, 512], F32R, tag="wv",
                                  name=f"wv{ch}_{k}")
                    nc.sync.dma_start(
                        out=wt,
                        in_=wv_ap[128 * k:128 * (k + 1),
                                  512 * ch:512 * (ch + 1)])
                    for tch in range(8):
                        nc.tensor.matmul(
                            ps[tch], xts[k][:, 128 * tch:128 * (tch + 1)], wt,
                            start=(k == 0), stop=(k == KC - 1))
                for tch in range(8):
                    nc.scalar.copy(out=vv[tch][:, 512 * ch:512 * (ch + 1)],
                                   in_=ps[tch])


def _phase_b(nc, tc, qT, kT, vv, maskT, ones_col, yt_loc, yt_all):
    """Attention per head; each head's yT is AllGathered as it completes."""
    Exp = mybir.ActivationFunctionType.Exp
    mult = mybir.AluOpType.mult

    with tc.tile_pool(name="pt", bufs=10) as ptp, \
         tc.tile_pool(name="yt", bufs=3) as yt_pool, \
         tc.tile_pool(name="att_sm", bufs=2) as asm, \
         tc.tile_pool(name="ps_s", bufs=3, space="PSUM") as pss, \
         tc.tile_pool(name="ps_y", bufs=2, space="PSUM") as psy, \
         tc.tile_pool(name="ps_r", bufs=2, space="PSUM") as psr:
        for h in range(HPC):
            pts = []
            for j in range(8):
                pt = ptp.tile([128, T], F32R, tag="pT", name=f"pT{h}_{j}")
                pts.append(pt)
                off = 128 * j
                while off < T:
                    cw = min(512, T - off)
                    sp = pss.tile([128, 512], F32, tag="sT")
                    nc.tensor.matmul(
                        sp[:, :cw], kT[h][:, 128 * j:128 * (j + 1)],
                        qT[h][:, off:off + cw], start=True, stop=True)
                    nc.scalar.activation(
                        out=pt[:, off - 128 * j:off - 128 * j + cw],
                        in_=sp[:, :cw], func=Exp, scale=SCALE)
                    off += cw
                # causal mask on the diagonal block (tile-local cols 0:128)
                nc.vector.tensor_tensor(out=pt[:, 0:128], in0=pt[:, 0:128],
                                        in1=maskT, op=mult)

            yt = yt_pool.tile([128, T], F32R, tag="yT", name=f"yt{h}")
            for g in range(2):
                tq0 = 512 * g
                jmax = 4 * (g + 1)
                yp = psy.tile([128, 512], F32, tag="yp")
                rp = psr.tile([1, 512], F32, tag="rp")
                for j in range(jmax):
                    lo = max(tq0, 128 * j)          # first valid tq
                    w = tq0 + 512 - lo
                    rhs = pts[j][:, lo - 128 * j:lo - 128 * j + w]
                    vblk = vv[j][:, 128 * h:128 * (h + 1)]
                    nc.tensor.matmul(yp[:, lo - tq0:lo - tq0 + w], vblk, rhs,
                                     start=(j == 0), stop=(j == jmax - 1))
                    nc.tensor.matmul(rp[:, lo - tq0:lo - tq0 + w], ones_col,
                                     rhs, start=(j == 0), stop=(j == jmax - 1))
                # softmax denominator: psum -> sbuf -> bcast -> recip -> mult
                r_sb = asm.tile([1, 512], F32, tag="r_sb")
                nc.vector.tensor_copy(out=r_sb, in_=rp)
                r_bc = asm.tile([128, 512], F32, tag="r_bc")
                nc.gpsimd.partition_broadcast(r_bc, r_sb)
                rec = asm.tile([128, 512], F32, tag="rec")
                nc.vector.reciprocal(out=rec, in_=r_bc)
                nc.vector.tensor_tensor(out=yt[:, tq0:tq0 + 512],
                                        in0=yp, in1=rec, op=mult)
            # ship this head's yT to the pair as soon as it's done
            nc.sync.dma_start(out=yt_loc[h], in_=yt)
            nc.gpsimd.collective_compute(
                "AllGather", mybir.AluOpType.bypass, replica_groups=PAIRS,
                ins=[yt_loc[h].opt()], outs=[yt_all[h].opt()])


def _phase_c(nc, tc, yt_all, wp_ap, out_ap):
    """Output projection out[t, c_half] = yT_full-gemm @ wp columns."""
    with tc.tile_pool(name="peer", bufs=2 * HPC) as peer_pool, \
         tc.tile_pool(name="wp", bufs=4) as wpp, \
         tc.tile_pool(name="out_sb", bufs=4) as osb, \
         tc.tile_pool(name="ps_o", bufs=8, space="PSUM") as pso:
        # Both gathered halves are loaded so the program is core-independent
        # (which half is "mine" differs per core; SPMD must not branch).
        yfull = []
        for r in range(TP):
            for h2 in range(HPC):
                t2 = peer_pool.tile([128, T], F32R, tag="yfull",
                                    name=f"yfull{r}_{h2}")
                nc.sync.dma_start(out=t2, in_=yt_all[h2][r])
                yfull.append(t2)

        for cc in range(2):          # 512-wide halves of my CPC out cols
            ps = [pso.tile([128, 512], F32, tag="po", name=f"po{cc}_{m}")
                  for m in range(8)]
            for kk in range(KC):
                wt = wpp.tile([128, 512], F32R, tag="wp",
                              name=f"wp{cc}_{kk}")
                nc.sync.dma_start(
                    out=wt, in_=wp_ap[128 * kk:128 * (kk + 1),
                                      512 * cc:512 * (cc + 1)])
                for m in range(8):
                    nc.tensor.matmul(
                        ps[m], yfull[kk][:, 128 * m:128 * (m + 1)], wt,
                        start=(kk == 0), stop=(kk == KC - 1))
            for m in range(8):
                ot = osb.tile([128, 512], F32, tag="ot")
                nc.scalar.copy(out=ot, in_=ps[m])
                nc.sync.dma_start(
                    out=out_ap[128 * m:128 * (m + 1),
                               512 * cc:512 * (cc + 1)],
                    in_=ot)


_NC_CACHE = None


def _get_nc():
    global _NC_CACHE
    if _NC_CACHE is None:
        _NC_CACHE = build_kernel()
    return _NC_CACHE


def kernel(x, w_qkv, w_proj, _trace=False, _trace_kwargs=None):
    x = np.asarray(x, dtype=np.float32)
    w_qkv = np.asarray(w_qkv, dtype=np.float32)
    w_proj = np.asarray(w_proj, dtype=np.float32)

    maskT = np.triu(np.ones((128, 128), dtype=np.float32))

    in_maps = []
    for c in range(N_CORES):
        b, hh = c // TP, c % TP
        cols = slice(hh * CPC, (hh + 1) * CPC)
        in_maps.append({
            "xT": np.ascontiguousarray(x[b].T),
            "wq": np.ascontiguousarray(w_qkv[:, :D][:, cols]),
            "wk": np.ascontiguousarray(w_qkv[:, D:2 * D][:, cols]),
            "wv": np.ascontiguousarray(w_qkv[:, 2 * D:][:, cols]),
            "wp": np.ascontiguousarray(w_proj[:, cols]),
            "maskT": maskT,
        })

    nc = _get_nc()
    res = run_bass_kernel_spmd(nc, in_maps, list(range(N_CORES)),
                               trace=_trace, **(_trace_kwargs or {}))

    out = np.empty((B, T, D), dtype=np.float32)
    for c in range(N_CORES):
        b, hh = c // TP, c % TP
        out[b, :, hh * CPC:(hh + 1) * CPC] = res.results[c]["out"]
    if _trace:
        return out, res
    return out


# revision 9
# speedup vs baseline: 1.6103x; 1.0146x over previous
"""Causal self-attention (B=4, T=1024, D=2048, H=16) on 8 trn2 NeuronCores.

Sharding: data-parallel over batch (4) x tensor-parallel over heads (2).
Core c handles batch b = c//2, head-half hh = c%2 (heads hh*8 .. hh*8+8).

Per-core plan (all matmuls float32r, fp32 PSUM accumulation):
  v      [t, c]  : lhsT = xT tile [k,t], rhs = wv [k,c]   (first, all heads)
  then per head h (pipelined):
    qT/kT [d, t] : lhsT = w_{q,k} tile [k,c=h], rhs = xT [k,t]
    sT    [tk,tq]: lhsT = kT block, rhs = qT slice (causal: tq >= 128*j only)
    pT    = exp(scale * sT) via ACT (no max-subtraction; |scaled scores| ~ 6)
    diag blocks masked multiplicatively with an upper-triangular 0/1 mask
    yT    [d, tq] += v_j-gemm: lhsT = v block, rhs = pT block (PSUM accum)
    r     [1, tq] += ones^T @ pT (softmax row sums, same rhs stream)
    yT_norm = yT * bcast(1/r) (DVE copy -> GpSimd bcast -> DVE approx-recip)
    pairwise AllGather of this head's yT (overlaps later heads' compute)
  out    [t, c_half] = yT_full-gemm against this half's w_proj columns
Host side: slice/transpose inputs per core, concat outputs (pure gather).
"""

import numpy as np

import concourse.bass as bass
import concourse.mybir as mybir
import concourse.tile as tile
from concourse import bacc
from concourse.bass_utils import run_bass_kernel_spmd

B, T, D = 4, 1024, 2048
H, DH = 16, 128
N_CORES = 8
TP = 2                      # head-halves per batch
HPC = H // TP               # heads per core = 8
CPC = HPC * DH              # channels per core = 1024
KC = D // 128               # contraction chunks = 16
SCALE = 1.0 / float(np.sqrt(DH))

F32 = mybir.dt.float32
F32R = mybir.dt.float32r

PAIRS = [[2 * i, 2 * i + 1] for i in range(B)]


def build_kernel():
    nc = bacc.Bacc("TRN2", target_bir_lowering=False, debug=False,
                   num_devices=N_CORES)

    xT_ap = nc.dram_tensor("xT", [D, T], F32R, kind="ExternalInput").ap()
    wq_ap = nc.dram_tensor("wq", [D, CPC], F32R, kind="ExternalInput").ap()
    wk_ap = nc.dram_tensor("wk", [D, CPC], F32R, kind="ExternalInput").ap()
    wv_ap = nc.dram_tensor("wv", [D, CPC], F32R, kind="ExternalInput").ap()
    wp_ap = nc.dram_tensor("wp", [D, CPC], F32R, kind="ExternalInput").ap()
    maskT_ap = nc.dram_tensor("maskT", [128, 128], F32R, kind="ExternalInput").ap()
    out_ap = nc.dram_tensor("out", [T, CPC], F32, kind="ExternalOutput").ap()

    with tile.TileContext(nc) as tc:
        _body(nc, tc, xT_ap, wq_ap, wk_ap, wv_ap, wp_ap, maskT_ap, out_ap)
    nc.compile()
    return nc


def _body(nc, tc, xT_ap, wq_ap, wk_ap, wv_ap, wp_ap, maskT_ap, out_ap):
    Exp = mybir.ActivationFunctionType.Exp
    mult = mybir.AluOpType.mult

    with tc.tile_pool(name="const", bufs=1) as const, \
         tc.tile_pool(name="dram", bufs=HPC, space="DRAM") as dram:
        maskT = const.tile([128, 128], F32R, tag="maskT")
        nc.sync.dma_start(out=maskT, in_=maskT_ap)
        ones_f32 = const.tile([128, 1], F32, tag="ones_f32")
        nc.vector.memset(ones_f32, 1.0)
        ones_col = const.tile([128, 1], F32R, tag="ones_col")
        nc.scalar.copy(out=ones_col, in_=ones_f32)

        yt_loc = [dram.tile([128, T], F32R, tag="ytl", name=f"ytl{h}")
                  for h in range(HPC)]
        yt_all = [dram.tile([TP, 128, T], F32R, tag="yta", name=f"yta{h}")
                  for h in range(HPC)]

        with tc.tile_pool(name="xa", bufs=KC) as xa, \
             tc.tile_pool(name="vvp", bufs=8) as vvp:
            xts = []
            for k in range(KC):
                xt = xa.tile([128, T], F32R, tag="xT", name=f"xt{k}")
                nc.sync.dma_start(out=xt, in_=xT_ap[128 * k:128 * (k + 1), :])
                xts.append(xt)
            vv = [vvp.tile([128, CPC], F32R, tag="vv", name=f"vv{j}")
                  for j in range(8)]

            # ---- v natural [t, c]: stationary xT slices, moving wv ----
            with tc.tile_pool(name="wv", bufs=3) as wvp, \
                 tc.tile_pool(name="pv", bufs=8, space="PSUM") as pv:
                for ch in range(2):
                    ps = [pv.tile([128, 512], F32, tag="pv",
                                  name=f"pv{ch}_{i}") for i in range(8)]
                    for k in range(KC):
                        wt = wvp.tile([128, 512], F32R, tag="wv",
                                      name=f"wv{ch}_{k}")
                        nc.sync.dma_start(
                            out=wt,
                            in_=wv_ap[128 * k:128 * (k + 1),
                                      512 * ch:512 * (ch + 1)])
                        for tch in range(8):
                            nc.tensor.matmul(
                                ps[tch],
                                xts[k][:, 128 * tch:128 * (tch + 1)], wt,
                                start=(k == 0), stop=(k == KC - 1))
                    for tch in range(8):
                        nc.scalar.copy(
                            out=vv[tch][:, 512 * ch:512 * (ch + 1)],
                            in_=ps[tch])

            # ---- per-head: qk gemm + attention + per-head AllGather ----
            with tc.tile_pool(name="wqk", bufs=3) as wqk, \
                 tc.tile_pool(name="qkp", bufs=4) as qkp, \
                 tc.tile_pool(name="pt", bufs=10) as ptp, \
                 tc.tile_pool(name="yt", bufs=2) as yt_pool, \
                 tc.tile_pool(name="att_sm", bufs=2) as asm, \
                 tc.tile_pool(name="pa", bufs=2, space="PSUM") as pa, \
                 tc.tile_pool(name="ps_s", bufs=2, space="PSUM") as pss, \
                 tc.tile_pool(name="ps_y", bufs=2, space="PSUM") as psy, \
                 tc.tile_pool(name="ps_r", bufs=2, space="PSUM") as psr:
                for h in range(HPC):
                    qkT = []
                    for (w_ap, nm) in ((wq_ap, "q"), (wk_ap, "k")):
                        outT = qkp.tile([128, T], F32R, tag="qkT",
                                        name=f"{nm}T{h}")
                        qkT.append(outT)
                        wt = wqk.tile([128, KC, 128], F32R, tag="wqk",
                                      name=f"w{nm}{h}")
                        nc.scalar.dma_start(
                            out=wt,
                            in_=w_ap[:, 128 * h:128 * (h + 1)].rearrange(
                                "(k p) j -> p k j", p=128))
                        for th in range(2):
                            ps = pa.tile([128, 512], F32, tag="pqk")
                            for k in range(KC):
                                nc.tensor.matmul(
                                    ps, wt[:, k, :],
                                    xts[k][:, 512 * th:512 * (th + 1)],
                                    start=(k == 0), stop=(k == KC - 1))
                            nc.scalar.copy(
                                out=outT[:, 512 * th:512 * (th + 1)], in_=ps)
                    qTh, kTh = qkT

                    pts = []
                    for j in range(8):
                        pt = ptp.tile([128, T], F32R, tag="pT",
                                      name=f"pT{h}_{j}")
                        pts.append(pt)
                        off = 128 * j
                        while off < T:
                            cw = min(512, T - off)
                            sp = pss.tile([128, 512], F32, tag="sT")
                            nc.tensor.matmul(
                                sp[:, :cw], kTh[:, 128 * j:128 * (j + 1)],
                                qTh[:, off:off + cw], start=True, stop=True)
                            nc.scalar.activation(
                                out=pt[:, off - 128 * j:off - 128 * j + cw],
                                in_=sp[:, :cw], func=Exp, scale=SCALE)
                            off += cw
                        # causal mask on the diagonal block (local cols 0:128)
                        nc.vector.tensor_tensor(
                            out=pt[:, 0:128], in0=pt[:, 0:128], in1=maskT,
                            op=mult)

                    yt = yt_pool.tile([128, T], F32R, tag="yT", name=f"yt{h}")
                    for g in range(2):
                        tq0 = 512 * g
                        jmax = 4 * (g + 1)
                        yp = psy.tile([128, 512], F32, tag="yp")
                        rp = psr.tile([1, 512], F32, tag="rp")
                        for j in range(jmax):
                            lo = max(tq0, 128 * j)          # first valid tq
                            w = tq0 + 512 - lo
                            rhs = pts[j][:, lo - 128 * j:lo - 128 * j + w]
                            vblk = vv[j][:, 128 * h:128 * (h + 1)]
                            nc.tensor.matmul(
                                yp[:, lo - tq0:lo - tq0 + w], vblk, rhs,
                                start=(j == 0), stop=(j == jmax - 1))
                            nc.tensor.matmul(
                                rp[:, lo - tq0:lo - tq0 + w], ones_col, rhs,
                                start=(j == 0), stop=(j == jmax - 1))
                        # softmax denom: psum -> sbuf -> bcast -> recip -> mult
                        r_sb = asm.tile([1, 512], F32, tag="r_sb")
                        nc.vector.tensor_copy(out=r_sb, in_=rp)
                        r_bc = asm.tile([128, 512], F32, tag="r_bc")
                        nc.gpsimd.partition_broadcast(r_bc, r_sb)
                        rec = asm.tile([128, 512], F32, tag="rec")
                        nc.vector.reciprocal_approx_fast(out=rec, in_=r_bc)
                        from concourse.dve_ops import RECIPROCAL_APPROX_NR
                        nc.vector._custom_dve(
                            RECIPROCAL_APPROX_NR, out=rec, in0=r_bc, in1=rec,
                            s0=2.0)
                        nc.vector.tensor_tensor(out=yt[:, tq0:tq0 + 512],
                                                in0=yp, in1=rec, op=mult)
                    # ship this head's yT to the pair as soon as it's done
                    nc.sync.dma_start(out=yt_loc[h], in_=yt)
                    nc.gpsimd.collective_compute(
                        "AllGather", mybir.AluOpType.bypass,
                        replica_groups=PAIRS,
                        ins=[yt_loc[h].opt()], outs=[yt_all[h].opt()])

            # ---- output projection out[t, c_half] = yT_full @ wp cols ----
            with tc.tile_pool(name="peer", bufs=2 * HPC) as peer_pool, \
                 tc.tile_pool(name="wp", bufs=4) as wpp, \
                 tc.tile_pool(name="out_sb", bufs=4) as osb, \
                 tc.tile_pool(name="ps_o", bufs=8, space="PSUM") as pso:
                yfull = []
                for r in range(TP):
                    for h2 in range(HPC):
                        t2 = peer_pool.tile([128, T], F32R, tag="yfull",
                                            name=f"yfull{r}_{h2}")
                        nc.sync.dma_start(out=t2, in_=yt_all[h2][r])
                        yfull.append(t2)
                for cc in range(2):      # 512-wide halves of my CPC out cols
                    ps = [pso.tile([128, 512], F32, tag="po",
                                   name=f"po{cc}_{m}") for m in range(8)]
                    for kk in range(KC):
                        wt = wpp.tile([128, 512], F32R, tag="wp",
                                      name=f"wp{cc}_{kk}")
                        nc.sync.dma_start(
                            out=wt, in_=wp_ap[128 * kk:128 * (kk + 1),
                                              512 * cc:512 * (cc + 1)])
                        for m in range(8):
                            nc.tensor.matmul(
                                ps[m], yfull[kk][:, 128 * m:128 * (m + 1)],
                                wt, start=(kk == 0), stop=(kk == KC - 1))
                    for m in range(8):
                        ot = osb.tile([128, 512], F32, tag="ot")
                        nc.scalar.copy(out=ot, in_=ps[m])
                        nc.sync.dma_start(
                            out=out_ap[128 * m:128 * (m + 1),
                                       512 * cc:512 * (cc + 1)],
                            in_=ot)


_NC_CACHE = None


def _get_nc():
    global _NC_CACHE
    if _NC_CACHE is None:
        _NC_CACHE = build_kernel()
    return _NC_CACHE


def kernel(x, w_qkv, w_proj, _trace=False, _trace_kwargs=None):
    x = np.asarray(x, dtype=np.float32)
    w_qkv = np.asarray(w_qkv, dtype=np.float32)
    w_proj = np.asarray(w_proj, dtype=np.float32)

    maskT = np.triu(np.ones((128, 128), dtype=np.float32))

    in_maps = []
    for c in range(N_CORES):
        b, hh = c // TP, c % TP
        cols = slice(hh * CPC, (hh + 1) * CPC)
        in_maps.append({
            "xT": np.ascontiguousarray(x[b].T),
            "wq": np.ascontiguousarray(w_qkv[:, :D][:, cols]),
            "wk": np.ascontiguousarray(w_qkv[:, D:2 * D][:, cols]),
            "wv": np.ascontiguousarray(w_qkv[:, 2 * D:][:, cols]),
            "wp": np.ascontiguousarray(w_proj[:, cols]),
            "maskT": maskT,
        })

    nc = _get_nc()
    res = run_bass_kernel_spmd(nc, in_maps, list(range(N_CORES)),
                               trace=_trace, **(_trace_kwargs or {}))

    out = np.empty((B, T, D), dtype=np.float32)
    for c in range(N_CORES):
        b, hh = c // TP, c % TP
        out[b, :, hh * CPC:(hh + 1) * CPC] = res.results[c]["out"]
    if _trace:
        return out, res
    return out


# revision 11
# speedup vs baseline: 1.9367x; 1.2027x over previous
"""Causal self-attention (B=4, T=1024, D=2048, H=16) on 8 trn2 NeuronCores.

Sharding: data-parallel over batch (4) x tensor-parallel over heads (2).
Core c handles batch b = c//2, head-half hh = c%2 (heads hh*8 .. hh*8+8).

Per-core plan (all matmuls float32r, fp32 PSUM accumulation):
  v      [t, c]  : lhsT = xT tile [k,t], rhs = wv [k,c]   (first, all heads)
  then per head h (pipelined):
    qT/kT [d, t] : lhsT = w_{q,k} tile [k,c=h], rhs = xT [k,t]
    sT    [tk,tq]: lhsT = kT block, rhs = qT slice (causal: tq >= 128*j only)
    pT    = exp(scale * sT) via ACT (no max-subtraction; |scaled scores| ~ 6)
    diag blocks masked multiplicatively with an upper-triangular 0/1 mask
    yT    [d, tq] += v_j-gemm: lhsT = v block, rhs = pT block (PSUM accum)
    r     [1, tq] += ones^T @ pT (softmax row sums, same rhs stream)
    yT_norm = yT * bcast(1/r) (DVE copy -> GpSimd bcast -> DVE approx-recip)
    pairwise AllGather of this head's yT (overlaps later heads' compute)
  out    [t, c_half] = yT_full-gemm against this half's w_proj columns
Host side: slice/transpose inputs per core, concat outputs (pure gather).
"""

import numpy as np

import concourse.bass as bass
import concourse.mybir as mybir
import concourse.tile as tile
from concourse import bacc
from concourse.bass_utils import run_bass_kernel_spmd

B, T, D = 4, 1024, 2048
H, DH = 16, 128
N_CORES = 8
TP = 2                      # head-halves per batch
HPC = H // TP               # heads per core = 8
CPC = HPC * DH              # channels per core = 1024
KC = D // 128               # contraction chunks = 16
SCALE = 1.0 / float(np.sqrt(DH))

F32 = mybir.dt.float32
F32R = mybir.dt.float32r

PAIRS = [[2 * i, 2 * i + 1] for i in range(B)]


def build_kernel():
    nc = bacc.Bacc("TRN2", target_bir_lowering=False, debug=False,
                   num_devices=N_CORES)

    xT_ap = nc.dram_tensor("xT", [D, T], F32R, kind="ExternalInput").ap()
    wq_ap = nc.dram_tensor("wq", [D, CPC], F32R, kind="ExternalInput").ap()
    wk_ap = nc.dram_tensor("wk", [D, CPC], F32R, kind="ExternalInput").ap()
    wv_ap = nc.dram_tensor("wv", [D, CPC], F32R, kind="ExternalInput").ap()
    wp_ap = nc.dram_tensor("wp", [D, CPC], F32R, kind="ExternalInput").ap()
    maskT_ap = nc.dram_tensor("maskT", [128, 128], F32R, kind="ExternalInput").ap()
    out_ap = nc.dram_tensor("out", [T, CPC], F32, kind="ExternalOutput").ap()

    with tile.TileContext(nc) as tc:
        _body(nc, tc, xT_ap, wq_ap, wk_ap, wv_ap, wp_ap, maskT_ap, out_ap)
    nc.compile()
    return nc


def _body(nc, tc, xT_ap, wq_ap, wk_ap, wv_ap, wp_ap, maskT_ap, out_ap):
    Exp = mybir.ActivationFunctionType.Exp
    mult = mybir.AluOpType.mult

    with tc.tile_pool(name="const", bufs=1) as const, \
         tc.tile_pool(name="dram", bufs=HPC, space="DRAM") as dram:
        maskT = const.tile([128, 128], F32R, tag="maskT")
        nc.sync.dma_start(out=maskT, in_=maskT_ap)
        ones_f32 = const.tile([128, 1], F32, tag="ones_f32")
        nc.vector.memset(ones_f32, 1.0)
        ones_col = const.tile([128, 1], F32R, tag="ones_col")
        nc.scalar.copy(out=ones_col, in_=ones_f32)

        yt_loc = [dram.tile([128, T], F32R, tag="ytl", name=f"ytl{h}")
                  for h in range(HPC)]
        yt_all = [dram.tile([TP, 128, T], F32R, tag="yta", name=f"yta{h}")
                  for h in range(HPC)]

        with tc.tile_pool(name="xa", bufs=KC) as xa, \
             tc.tile_pool(name="vvp", bufs=8) as vvp:
            xts = []
            for k in range(KC):
                xt = xa.tile([128, T], F32R, tag="xT", name=f"xt{k}")
                nc.sync.dma_start(out=xt, in_=xT_ap[128 * k:128 * (k + 1), :])
                xts.append(xt)
            vv = [vvp.tile([128, CPC], F32R, tag="vv", name=f"vv{j}")
                  for j in range(8)]

            # ---- v natural [t, c]: stationary xT slices, moving wv ----
            with tc.tile_pool(name="wv", bufs=3) as wvp, \
                 tc.tile_pool(name="pv", bufs=8, space="PSUM") as pv:
                for ch in range(2):
                    ps = [pv.tile([128, 512], F32, tag="pv",
                                  name=f"pv{ch}_{i}") for i in range(8)]
                    for k in range(KC):
                        wt = wvp.tile([128, 512], F32R, tag="wv",
                                      name=f"wv{ch}_{k}")
                        nc.scalar.dma_start(
                            out=wt,
                            in_=wv_ap[128 * k:128 * (k + 1),
                                      512 * ch:512 * (ch + 1)])
                        for tch in range(8):
                            nc.tensor.matmul(
                                ps[tch],
                                xts[k][:, 128 * tch:128 * (tch + 1)], wt,
                                start=(k == 0), stop=(k == KC - 1))
                    for tch in range(8):
                        nc.scalar.copy(
                            out=vv[tch][:, 512 * ch:512 * (ch + 1)],
                            in_=ps[tch])

            # ---- per-head: qk gemm + attention + per-head AllGather ----
            with tc.tile_pool(name="wqk", bufs=4) as wqk, \
                 tc.tile_pool(name="qkp", bufs=4) as qkp, \
                 tc.tile_pool(name="pt", bufs=10) as ptp, \
                 tc.tile_pool(name="yt", bufs=2) as yt_pool, \
                 tc.tile_pool(name="att_sm", bufs=2) as asm, \
                 tc.tile_pool(name="pa", bufs=2, space="PSUM") as pa, \
                 tc.tile_pool(name="ps_s", bufs=2, space="PSUM") as pss, \
                 tc.tile_pool(name="ps_y", bufs=2, space="PSUM") as psy, \
                 tc.tile_pool(name="ps_r", bufs=2, space="PSUM") as psr:
                def load_wqk(h2):
                    tiles = []
                    for (w_ap, nm) in ((wq_ap, "q"), (wk_ap, "k")):
                        wt = wqk.tile([128, KC, 128], F32R, tag="wqk",
                                      name=f"w{nm}{h2}")
                        nc.sync.dma_start(
                            out=wt,
                            in_=w_ap[:, 128 * h2:128 * (h2 + 1)].rearrange(
                                "(k p) j -> p k j", p=128))
                        tiles.append(wt)
                    return tiles

                wts_next = load_wqk(0)
                for h in range(HPC):
                    wts_cur = wts_next
                    if h + 1 < HPC:
                        wts_next = load_wqk(h + 1)
                    qkT = []
                    for wi, nm in ((0, "q"), (1, "k")):
                        outT = qkp.tile([128, T], F32R, tag="qkT",
                                        name=f"{nm}T{h}")
                        qkT.append(outT)
                        wt = wts_cur[wi]
                        for th in range(2):
                            ps = pa.tile([128, 512], F32, tag="pqk")
                            for k in range(KC):
                                nc.tensor.matmul(
                                    ps, wt[:, k, :],
                                    xts[k][:, 512 * th:512 * (th + 1)],
                                    start=(k == 0), stop=(k == KC - 1))
                            nc.scalar.copy(
                                out=outT[:, 512 * th:512 * (th + 1)], in_=ps)
                    qTh, kTh = qkT

                    pts = []
                    for j in range(8):
                        pt = ptp.tile([128, T], F32R, tag="pT",
                                      name=f"pT{h}_{j}")
                        pts.append(pt)
                        off = 128 * j
                        while off < T:
                            cw = min(512, T - off)
                            sp = pss.tile([128, 512], F32, tag="sT")
                            nc.tensor.matmul(
                                sp[:, :cw], kTh[:, 128 * j:128 * (j + 1)],
                                qTh[:, off:off + cw], start=True, stop=True)
                            nc.scalar.activation(
                                out=pt[:, off - 128 * j:off - 128 * j + cw],
                                in_=sp[:, :cw], func=Exp, scale=SCALE)
                            off += cw
                        # causal mask on the diagonal block (local cols 0:128)
                        nc.vector.tensor_tensor(
                            out=pt[:, 0:128], in0=pt[:, 0:128], in1=maskT,
                            op=mult)

                    yt = yt_pool.tile([128, T], F32R, tag="yT", name=f"yt{h}")
                    for g in range(2):
                        tq0 = 512 * g
                        jmax = 4 * (g + 1)
                        yp = psy.tile([128, 512], F32, tag="yp")
                        rp = psr.tile([1, 512], F32, tag="rp")
                        for j in range(jmax):
                            lo = max(tq0, 128 * j)          # first valid tq
                            w = tq0 + 512 - lo
                            rhs = pts[j][:, lo - 128 * j:lo - 128 * j + w]
                            vblk = vv[j][:, 128 * h:128 * (h + 1)]
                            nc.tensor.matmul(
                                yp[:, lo - tq0:lo - tq0 + w], vblk, rhs,
                                start=(j == 0), stop=(j == jmax - 1))
                            nc.tensor.matmul(
                                rp[:, lo - tq0:lo - tq0 + w], ones_col, rhs,
                                start=(j == 0), stop=(j == jmax - 1))
                        # softmax denom: psum -> sbuf -> bcast -> recip -> mult
                        r_sb = asm.tile([1, 512], F32, tag="r_sb")
                        nc.vector.tensor_copy(out=r_sb, in_=rp)
                        r_bc = asm.tile([128, 512], F32, tag="r_bc")
                        nc.gpsimd.partition_broadcast(r_bc, r_sb)
                        rec = asm.tile([128, 512], F32, tag="rec")
                        nc.vector.reciprocal_approx_fast(out=rec, in_=r_bc)
                        from concourse.dve_ops import RECIPROCAL_APPROX_NR
                        nc.vector._custom_dve(
                            RECIPROCAL_APPROX_NR, out=rec, in0=r_bc, in1=rec,
                            s0=2.0)
                        nc.vector.tensor_tensor(out=yt[:, tq0:tq0 + 512],
                                                in0=yp, in1=rec, op=mult)
                    # ship this head's yT to the pair as soon as it's done
                    nc.sync.dma_start(out=yt_loc[h], in_=yt)
                    nc.gpsimd.collective_compute(
                        "AllGather", mybir.AluOpType.bypass,
                        replica_groups=PAIRS,
                        ins=[yt_loc[h].opt()], outs=[yt_all[h].opt()])

            # ---- output projection out[t, c_half] = yT_full @ wp cols ----
            with tc.tile_pool(name="peer", bufs=2 * HPC) as peer_pool, \
                 tc.tile_pool(name="wp", bufs=4) as wpp, \
                 tc.tile_pool(name="out_sb", bufs=4) as osb, \
                 tc.tile_pool(name="ps_o", bufs=8, space="PSUM") as pso:
                yfull = []
                for r in range(TP):
                    for h2 in range(HPC):
                        t2 = peer_pool.tile([128, T], F32R, tag="yfull",
                                            name=f"yfull{r}_{h2}")
                        nc.sync.dma_start(out=t2, in_=yt_all[h2][r])
                        yfull.append(t2)
                for cc in range(2):      # 512-wide halves of my CPC out cols
                    ps = [pso.tile([128, 512], F32, tag="po",
                                   name=f"po{cc}_{m}") for m in range(8)]
                    for kk in range(KC):
                        wt = wpp.tile([128, 512], F32R, tag="wp",
                                      name=f"wp{cc}_{kk}")
                        nc.scalar.dma_start(
                            out=wt, in_=wp_ap[128 * kk:128 * (kk + 1),
                                              512 * cc:512 * (cc + 1)])
                        for m in range(8):
                            nc.tensor.matmul(
                                ps[m], yfull[kk][:, 128 * m:128 * (m + 1)],
                                wt, start=(kk == 0), stop=(kk == KC - 1))
                    for m in range(8):
                        ot = osb.tile([128, 512], F32, tag="ot")
                        nc.scalar.copy(out=ot, in_=ps[m])
                        nc.sync.dma_start(
                            out=out_ap[128 * m:128 * (m + 1),
                                       512 * cc:512 * (cc + 1)],
                            in_=ot)


_NC_CACHE = None


def _get_nc():
    global _NC_CACHE
    if _NC_CACHE is None:
        _NC_CACHE = build_kernel()
    return _NC_CACHE


def kernel(x, w_qkv, w_proj, _trace=False, _trace_kwargs=None):
    x = np.asarray(x, dtype=np.float32)
    w_qkv = np.asarray(w_qkv, dtype=np.float32)
    w_proj = np.asarray(w_proj, dtype=np.float32)

    maskT = np.triu(np.ones((128, 128), dtype=np.float32))

    in_maps = []
    for c in range(N_CORES):
        b, hh = c // TP, c % TP
        cols = slice(hh * CPC, (hh + 1) * CPC)
        in_maps.append({
            "xT": np.ascontiguousarray(x[b].T),
            "wq": np.ascontiguousarray(w_qkv[:, :D][:, cols]),
            "wk": np.ascontiguousarray(w_qkv[:, D:2 * D][:, cols]),
            "wv": np.ascontiguousarray(w_qkv[:, 2 * D:][:, cols]),
            "wp": np.ascontiguousarray(w_proj[:, cols]),
            "maskT": maskT,
        })

    nc = _get_nc()
    res = run_bass_kernel_spmd(nc, in_maps, list(range(N_CORES)),
                               trace=_trace, **(_trace_kwargs or {}))

    out = np.empty((B, T, D), dtype=np.float32)
    for c in range(N_CORES):
        b, hh = c // TP, c % TP
        out[b, :, hh * CPC:(hh + 1) * CPC] = res.results[c]["out"]
    if _trace:
        return out, res
    return out
